# revision 3
# baseline (speedup 1.0000x reference)
"""Trainium2 Bass kernel v2: causal sliding-window attention block.

Model (see reference): x:[2,2048,512] -> q/k/v proj (8 heads x 64) ->
causal sliding-window attention (W=128) -> out proj.

Sharding: 8 cores = 2 batches x 4 sequence chunks of 512 rows. Each core
gets a 640-row halo slice of x, all four weight matrices, computes its
512x512 output chunk. No cross-core communication.

v2 redesign vs v1 (driven by the CoreSim-v1 cost model):
  * Projections as fp8e4 DoubleRow matmuls (K=256/instr, 0.5 cyc/row):
    x and W host-pre-split into (hi, lo) fp8 pairs;
    W@x ~= Whi@xhi + Whi@xlo + Wlo@xhi (dropping lo@lo; ~1e-3 rel err,
    better than bf16). 25% fewer PE cycles than bf16.
  * Attention is query-block oriented: per (pair, qblock j) scores tile
    [128w, 512] = [hbj_e | hbj_o | hbj+1_e | hbj+1_o]; one exp, one mask
    multiply, then:
      - softmax denominators via N=1 matmuls (et stationary, ones
        moving) accumulated into a per-pair [128q, 8] PSUM region
        (~free on PE vs 2048 cyc/pair of one-hot matmuls);
      - AV with v stationary / et moving, head halves col-packed into
        one [128, 512] oT bank.
  * Normalize without DRAM bounce: reciprocal of [128, 8] sums (one
    cheap DVE op), PE transposes (identity trick) to [2, 128] rows,
    per-qblock K=2 broadcast matmuls build rbc [128dh, 512q], one DVE
    multiply normalizes the pair.
  * No zero-fill matmuls: PE executes in queue order, so each
    accumulation region's first matmul carries start=True (overwrite).
  * Input DMAs spread across SP/DVE/Pool with host-prepacked
    partition-contiguous layouts; output DMAs spread across engines.
"""

from contextlib import ExitStack

import numpy as np
import ml_dtypes

import concourse.bacc as bacc
import concourse.tile as tile
import concourse.mybir as mybir
from concourse import bass_utils

BF16 = mybir.dt.bfloat16
F32 = mybir.dt.float32
F8 = mybir.dt.float8e4
DRMODE = mybir.MatmulPerfMode.DoubleRow

P = 128          # partitions / block size / window
S = 512          # chunk rows per core
SH = 640         # halo rows per core (128 + 512)
D = 512          # d_model
DH = 64          # head dim
NKT = 4          # d_model tiles of 128
NST = 5          # halo row tiles of 128
NPAIR = 4        # head pairs
N_CORES = 8

E4 = ml_dtypes.float8_e4m3

_nc_cache = None
_DEBUG = False


def _build_kernel():
    nc = bacc.Bacc("TRN2", target_bir_lowering=False, debug=False,
                   enable_asserts=False)

    # x8: [128, 2(hi/lo) x 4(ktile) x 640] fp8; ktile k row p holds
    # x[halo_row, 128k + p].
    x8_d = nc.dram_tensor("x8", [P, 2 * NKT * SH], F8, kind="ExternalInput")
    wq8_d = nc.dram_tensor("wq8", [P, 2 * NKT * D], F8, kind="ExternalInput")
    wk8_d = nc.dram_tensor("wk8", [P, 2 * NKT * D], F8, kind="ExternalInput")
    wv8_d = nc.dram_tensor("wv8", [P, 2 * NKT * D], F8, kind="ExternalInput")
    wo_d = nc.dram_tensor("wo", [P, NKT * D], BF16, kind="ExternalInput")
    # masks blob [128, 1153] bf16:
    #   0:512     mask_mid = [prev|prev|curr|curr]
    #   512:1024  mask_q0  (qblock-0 variant; prev half zeroed on chunk 0)
    #   1024:1152 identity(128)
    #   1152:1153 ones column
    #   1153:1281 sel2 [2, 128] on rows 0:2
    masks_d = nc.dram_tensor("masks", [P, 1281], BF16, kind="ExternalInput")
    out_d = nc.dram_tensor("out", [S, D], BF16, kind="ExternalOutput")
    dbg = {}
    if _DEBUG:
        for nm, shp in [("qTe0", [P, S]), ("qTo0", [P, S]), ("kT0", [P, SH]),
                        ("v0", [P, S]), ("v1", [P, S]), ("et0", [P, S]),
                        ("sums0", [P, 8]), ("ot0", [P, S]), ("rbc0", [P, S]),
                        ("rinv0", [P, 8]), ("trsb0", [2, S]),
                        ("oT0", [P, S])]:
            dbg[nm] = nc.dram_tensor(nm, shp, F32, kind="ExternalOutput")

    with tile.TileContext(nc) as tc, ExitStack() as ctx:
        _kernel_body(ctx, tc, x8_d, wq8_d, wk8_d, wv8_d, wo_d, masks_d, out_d,
                     dbg)
    nc.compile()
    return nc


def _kernel_body(ctx, tc, x8_d, wq8_d, wk8_d, wv8_d, wo_d, masks_d, out_d,
                 dbg={}):
    nc = tc.nc
    Exp = mybir.ActivationFunctionType.Exp
    Copy = mybir.ActivationFunctionType.Copy

    persist = ctx.enter_context(tc.tile_pool(name="persist", bufs=1))
    expp = ctx.enter_context(tc.tile_pool(name="expp", bufs=3))
    outp = ctx.enter_context(tc.tile_pool(name="outp", bufs=4))
    pp = ctx.enter_context(tc.tile_pool(name="pp", bufs=2, space="PSUM"))
    psc = ctx.enter_context(tc.tile_pool(name="psc", bufs=3, space="PSUM"))
    poT = ctx.enter_context(tc.tile_pool(name="poT", bufs=2, space="PSUM"))
    pmisc = ctx.enter_context(tc.tile_pool(name="pmisc", bufs=1, space="PSUM"))

    sel_even = persist.tile([P, 1], F32, tag="sel_even")
    nc.gpsimd.memset(sel_even[:], 1.0)
    nc.gpsimd.memset(sel_even[DH:P, :], 0.0)
    sel_odd = persist.tile([P, 1], F32, tag="sel_odd")
    nc.gpsimd.memset(sel_odd[:], 0.0)
    nc.gpsimd.memset(sel_odd[DH:P, :], 1.0)

    # ---------------- input DMAs (spread across engines) ----------------
    x8 = persist.tile([P, 2 * NKT * SH], F8, tag="x8")
    wq8 = persist.tile([P, 2 * NKT * D], F8, tag="wq8")
    wk8 = persist.tile([P, 2 * NKT * D], F8, tag="wk8")
    wv8 = persist.tile([P, 2 * NKT * D], F8, tag="wv8")
    wo_sb = persist.tile([P, NKT * D], BF16, tag="wo")
    masks = persist.tile([P, 1281], BF16, tag="masks")

    # ACT carries x-hi (its table load is pushed behind this DMA by the
    # dummy activation below); SP: wq-hi then x-lo; Pool: wq-lo, wk, masks.
    nc.scalar.dma_start(x8[:, 0:2 * SH], x8_d.ap()[:, 0:2 * SH])
    nc.sync.dma_start(wq8[:, 0:2 * D], wq8_d.ap()[:, 0:2 * D])
    nc.scalar.dma_start(x8[:, 2 * SH:4 * SH], x8_d.ap()[:, 2 * SH:4 * SH])
    nc.sync.dma_start(wq8[:, 2 * D:NKT * D], wq8_d.ap()[:, 2 * D:NKT * D])
    nc.gpsimd.dma_start(wq8[:, NKT * D:], wq8_d.ap()[:, NKT * D:])
    nc.sync.dma_start(x8[:, 4 * SH:8 * SH], x8_d.ap()[:, 4 * SH:8 * SH])
    nc.gpsimd.dma_start(wk8[:], wk8_d.ap()[:])
    nc.gpsimd.dma_start(masks[:], masks_d.ap()[:])
    nc.sync.dma_start(wv8[:], wv8_d.ap()[:])
    nc.sync.dma_start(wo_sb[:], wo_d.ap()[:])

    # PE warmup: a 1x1 matmul as early as possible latches pe_busy_start
    # so the p-state ramp completes during the input-load stall.
    warm = pmisc.tile([P, 1024], BF16, tag="misc")
    warm_f32 = warm[:].bitcast(F32)
    nc.tensor.matmul(warm_f32[0:1, 40:41], sel_even[0:1, 0:1],
                     sel_even[0:1, 0:1], start=True, stop=True,
                     skip_group_check=True)

    # Dummy activation: the auto-inserted LoadActFuncSet attaches to the
    # first activation and inherits its waits; a no-dep dummy right after
    # the ACT-queue DMA pulls the 1283ns table load off the critical path.
    atl_scratch = persist.tile([P, 1], F32, tag="atl_scratch")
    nc.scalar.activation(atl_scratch[:], sel_even[:], Copy)

    mask_mid = masks[:, 0:512]
    mask_q0 = masks[:, 512:1024]
    ident = masks[:, 1024:1152]
    ones = masks[:, 1152:1153]
    sel2 = masks[0:2, 1153:1281]

    x8v = x8[:].rearrange("p (pt k s) -> p pt k s", pt=2, k=NKT)

    def x_dr(part, kp, c0, c1):
        # [128, 2, c1-c0]: ktile pair (2kp, 2kp+1), halo cols c0:c1
        return x8v[:, part, 2 * kp:2 * kp + 2, c0:c1]

    def w_dr(w8, part, kp, m0, m1):
        wv_ = w8[:].rearrange("p (pt k m) -> p pt k m", pt=2, k=NKT)
        return wv_[:, part, 2 * kp:2 * kp + 2, m0:m1]

    TERMS = [(0, 0), (1, 0), (0, 1)]   # (W part, x part): hi@hi, lo@hi, hi@lo

    def dr_wx(ps_ap, w8, m0, m1, c0, c1):
        """ps = W[:, m0:m1]^T @ xT[:, c0:c1] via 6 DoubleRow matmuls."""
        n = 0
        for wp, xp in TERMS:
            for kp in range(2):
                nc.tensor.matmul(ps_ap, w_dr(w8, wp, kp, m0, m1),
                                 x_dr(xp, kp, c0, c1),
                                 start=(n == 0), stop=(n == 5),
                                 perf_mode=DRMODE)
                n += 1

    def dr_xw(ps_ap, w8, c0, c1):
        """ps = x[c0:c1 rows] @ W via 6 DoubleRow matmuls."""
        n = 0
        for wp, xp in TERMS:
            for kp in range(2):
                nc.tensor.matmul(ps_ap, x_dr(xp, kp, c0, c1),
                                 w_dr(w8, wp, kp, 0, D),
                                 start=(n == 0), stop=(n == 5),
                                 perf_mode=DRMODE)
                n += 1

    # ---------------- projections ----------------
    qTe_sb = [persist.tile([P, S], BF16, tag=f"qTe{t}", name=f"qTe{t}")
              for t in range(NPAIR)]
    qTo_sb = [persist.tile([P, S], BF16, tag=f"qTo{t}", name=f"qTo{t}")
              for t in range(NPAIR)]
    kT_sb = [persist.tile([P, SH], BF16, tag=f"kT{t}", name=f"kT{t}")
             for t in range(NPAIR)]
    v_sb = [persist.tile([P, S], BF16, tag=f"v{st}", name=f"v{st}")
            for st in range(NST)]

    def q_proj(m, pool=None):
        ps = (pool or pp).tile([P, S], F32,
                               tag="pp" if pool is None else "sc",
                               name=f"qps{m}")
        dr_wx(ps[:], wq8, m * P, (m + 1) * P, P, SH)
        nc.scalar.activation(qTe_sb[m][:], ps[:], Copy, scale=sel_even[:])
        nc.vector.tensor_scalar_mul(qTo_sb[m][:], ps[:], sel_odd[:])

    def k_proj(m, pool=None):
        ps = (pool or pp).tile([P, S], F32,
                               tag="pp" if pool is None else "sc",
                               name=f"kps{m}")
        dr_wx(ps[:], wk8, m * P, (m + 1) * P, 0, S)
        nc.vector.tensor_copy(kT_sb[m][:, 0:S], ps[:])
        ps2 = pp.tile([P, S], F32, tag="pp", name=f"kpsh{m}")
        dr_wx(ps2[:, 0:P], wk8, m * P, (m + 1) * P, S, SH)
        nc.vector.tensor_copy(kT_sb[m][:, S:SH], ps2[:, 0:P])

    def v_proj(st):
        ps = pp.tile([P, S], F32, tag="pp", name=f"vps{st}")
        dr_xw(ps[:], wv8, st * P, (st + 1) * P)
        if st % 2 == 0:
            nc.vector.tensor_copy(v_sb[st][:], ps[:])
        else:
            nc.scalar.copy(v_sb[st][:], ps[:])

    # ---------------- attention ----------------
    # misc PSUM bank: bytes 0:128 = sums (4 pairs x [128, 8] f32),
    # bytes 1024:2048 = trT scratch [2, 512] bf16.
    misc = warm
    misc_f32 = warm_f32
    trT = misc[0:2, 512:1024]
    rbcq = misc_f32[:, 128:256]   # [128, 128] f32 slot for pair-3 chainlets

    ot_sb = [persist.tile([P, S], BF16, tag=f"ot{t}", name=f"ot{t}")
             for t in range(NPAIR)]

    def attn_scores(t, j):
        sc = psc.tile([P, S], F32, tag="sc", name=f"sc{t}_{j}")
        for col, (hb, qt) in enumerate(
                [(j, qTe_sb[t]), (j, qTo_sb[t]),
                 (j + 1, qTe_sb[t]), (j + 1, qTo_sb[t])]):
            nc.tensor.matmul(sc[:, col * P:(col + 1) * P],
                             kT_sb[t][:, hb * P:(hb + 1) * P],
                             qt[:, j * P:(j + 1) * P],
                             start=True, stop=True)
        return sc

    def attn_tail(t, j, sc, oT_ps, on_dve):
        et = expp.tile([P, S], BF16, tag="et", name=f"et{t}_{j}")
        # q,k carry a 16x host-side weight scale each: 0.125 / 256
        nc.scalar.activation(et[:], sc[:], Exp, scale=0.125 / 256.0)
        mask = mask_q0 if j == 0 else mask_mid
        nc.gpsimd.tensor_mul(et[:], et[:], mask[:])
        # sums: et slice stationary, ones moving -> [128q, 1] f32
        for h in range(2):
            for bi in range(2):
                c = 8 * t + 2 * j + h
                nc.tensor.matmul(misc_f32[:, c:c + 1],
                                 et[:, (2 * bi + h) * P:(2 * bi + h + 1) * P],
                                 ones[:],
                                 start=(bi == 0), stop=(bi == 1),
                                 skip_group_check=True)
        # AV: v stationary, et moving; head halves col-packed
        for h in range(2):
            for bi in range(2):
                nc.tensor.matmul(
                    oT_ps[h * DH:(h + 1) * DH, j * P:(j + 1) * P],
                    v_sb[j + bi][:, (2 * t + h) * DH:(2 * t + h + 1) * DH],
                    et[:, (2 * bi + h) * P:(2 * bi + h + 1) * P],
                    start=(bi == 0), stop=(bi == 1),
                    tile_position=(0, h * DH), skip_group_check=True)

    def attn_norm(t, oT_ps):
        # oT eviction is independent of the reciprocal chain and frees the
        # poT slot that rbc then reuses; the multiply reads SBUF x PSUM.
        ot_un = persist.tile([P, S], F32, tag="ot_un", name=f"ot_un{t}",
                             bufs=2)
        nc.vector.tensor_copy(ot_un[:], oT_ps[:])
        rinv = persist.tile([P, 8], BF16, tag="rinv", name=f"rinv{t}",
                            bufs=2)
        with nc.allow_low_precision(reason="1/sums in bf16: ~0.4% on the "
                                    "softmax scale, inside error budget"):
            nc.vector.reciprocal(rinv[:], misc_f32[:, 8 * t:8 * t + 8])
        # transpose per qblock: trT[:, j*128:(j+1)*128] = rinv[:, 2j:2j+2]^T
        for j in range(4):
            nc.tensor.transpose(trT[:, j * P:(j + 1) * P],
                                rinv[:, 2 * j:2 * j + 2], ident[:])
        trsb = persist.tile([2, S], BF16, tag="trsb", name=f"trsb{t}",
                            bufs=2)
        nc.vector.tensor_copy(trsb[:], trT[:])
        rbc = sc[t, 3]   # overwrite the pair's last scores tile
        for j in range(4):
            nc.tensor.matmul(rbc[:, j * P:(j + 1) * P],
                             sel2, trsb[:, j * P:(j + 1) * P],
                             start=True, stop=True)
        nc.vector.tensor_mul(ot_sb[t][:], ot_un[:], rbc[:])

    def attn_norm_qb(t, j, ot_un, rbc, rinv, trsb, meng):
        with nc.allow_low_precision(reason="1/sums in bf16, inside budget"):
            nc.vector.reciprocal(rinv[:, 2 * j:2 * j + 2],
                                 misc_f32[:, 8 * t + 2 * j:8 * t + 2 * j + 2])
        nc.tensor.transpose(trT[:, j * P:(j + 1) * P],
                            rinv[:, 2 * j:2 * j + 2], ident[:])
        nc.vector.tensor_copy(trsb[:, j * P:(j + 1) * P],
                              trT[:, j * P:(j + 1) * P])
        nc.tensor.matmul(rbcq, sel2, trsb[:, j * P:(j + 1) * P],
                         start=True, stop=True, skip_group_check=True)
        meng.tensor_mul(ot_sb[t][:, j * P:(j + 1) * P],
                        ot_un[:, j * P:(j + 1) * P],
                        rbcq)

    # ---------------- out projection ----------------
    def fmm(mt, t, f):
        nc.tensor.matmul(f[:], ot_sb[t][:, mt * P:(mt + 1) * P],
                         wo_sb[:, t * D:(t + 1) * D],
                         start=(t == 0), stop=(t == NPAIR - 1),
                         skip_group_check=True)

    def fout(mt, f, ceng, deng):
        osb = outp.tile([P, S], BF16, tag="osb", name=f"osb{mt}")
        if ceng is nc.scalar:
            nc.scalar.copy(osb[:], f[:])
        else:
            ceng.tensor_copy(osb[:], f[:])
        deng.dma_start(out_d.ap()[mt * P:(mt + 1) * P, :], osb[:])

    # ---------------- emission schedule ----------------
    # Flat software pipeline: scores run >=2 qblocks ahead of their
    # sums/AV consumers, with projection work woven into pair 0 and
    # out-projection accumulation woven into pairs 1-3, so the PE queue
    # never parks behind the exp->mask chain.
    q_proj(0)
    q_proj(1)
    k_proj(0, psc)
    q_proj(2, psc)
    q_proj(3, psc)
    v_proj(0)
    v_proj(1)

    fps = {}
    oT = {0: poT.tile([P, S], F32, tag="oT", name="oT0")}
    sc = {(0, 0): attn_scores(0, 0)}
    k_proj(1)
    sc[0, 1] = attn_scores(0, 1)
    attn_tail(0, 0, sc[0, 0], oT[0], False)
    v_proj(2)
    sc[0, 2] = attn_scores(0, 2)
    attn_tail(0, 1, sc[0, 1], oT[0], True)
    k_proj(2)
    v_proj(3)
    sc[0, 3] = attn_scores(0, 3)
    attn_tail(0, 2, sc[0, 2], oT[0], False)
    k_proj(3)
    v_proj(4)
    attn_tail(0, 3, sc[0, 3], oT[0], True)

    for t in (1, 2):
        oT[t] = poT.tile([P, S], F32, tag="oT", name=f"oT{t}")
        if t == 1:
            sc[t, 0] = attn_scores(t, 0)
            sc[t, 1] = attn_scores(t, 1)
        attn_norm(t - 1, oT[t - 1])   # PE pieces land between score bursts
        attn_tail(t, 0, sc[t, 0], oT[t], False)
        sc[t, 2] = attn_scores(t, 2)
        attn_tail(t, 1, sc[t, 1], oT[t], True)
        if t == 1:
            fps[0] = pp.tile([P, S], F32, tag="pp", name="fps0")
            fps[1] = pp.tile([P, S], F32, tag="pp", name="fps1")
        else:
            for mt in range(2):
                fmm(mt, 0, fps[mt])
        sc[t, 3] = attn_scores(t, 3)
        attn_tail(t, 2, sc[t, 2], oT[t], False)
        sc[t + 1, 0] = attn_scores(t + 1, 0)
        attn_tail(t, 3, sc[t, 3], oT[t], True)
        sc[t + 1, 1] = attn_scores(t + 1, 1)

    # pair 3: per-qblock normalize chains woven into the attention so the
    # out-projection + output DMA of m-tile j start as soon as qblock j
    # is normalized.
    rinv3 = persist.tile([P, 8], BF16, tag="rinv3", name="rinv3")
    trsb3 = persist.tile([2, S], BF16, tag="trsb3", name="trsb3")
    ot_un3 = persist.tile([P, S], F32, tag="ot_un", name="ot_un3", bufs=2)
    oT[3] = poT.tile([P, S], F32, tag="oT", name="oT3")
    attn_norm(2, oT[2])
    attn_tail(3, 0, sc[3, 0], oT[3], False)
    sc[3, 2] = attn_scores(3, 2)
    attn_tail(3, 1, sc[3, 1], oT[3], True)
    for mt in range(2):
        fmm(mt, 1, fps[mt])
    fps[2] = psc.tile([P, S], F32, tag="sc", name="fps2")
    for tt in range(3):
        fmm(2, tt, fps[2])
    nc.vector.tensor_copy(ot_un3[:, 0:P], oT[3][:, 0:P])
    attn_norm_qb(3, 0, ot_un3, None, rinv3, trsb3, nc.vector)
    sc[3, 3] = attn_scores(3, 3)
    attn_tail(3, 2, sc[3, 2], oT[3], False)
    nc.scalar.copy(ot_un3[:, P:2 * P], oT[3][:, P:2 * P])
    fps[3] = psc.tile([P, S], F32, tag="sc", name="fps3")
    for tt in range(3):
        fmm(3, tt, fps[3])
    for mt in range(2):
        fmm(mt, 2, fps[mt])
    attn_norm_qb(3, 1, ot_un3, None, rinv3, trsb3, nc.vector)
    fmm(0, 3, fps[0])
    fout(0, fps[0], nc.vector, nc.sync)
    attn_tail(3, 3, sc[3, 3], oT[3], True)
    nc.vector.tensor_copy(ot_un3[:, 2 * P:3 * P], oT[3][:, 2 * P:3 * P])
    nc.scalar.copy(ot_un3[:, 3 * P:4 * P], oT[3][:, 3 * P:4 * P])
    fmm(1, 3, fps[1])
    fout(1, fps[1], nc.scalar, nc.scalar)
    attn_norm_qb(3, 2, ot_un3, None, rinv3, trsb3, nc.vector)
    fmm(2, 3, fps[2])
    fout(2, fps[2], nc.scalar, nc.scalar)
    attn_norm_qb(3, 3, ot_un3, None, rinv3, trsb3, nc.vector)
    fmm(3, 3, fps[3])
    fout(3, fps[3], nc.vector, nc.sync)

    if dbg:
        def ddump(nm, ap):
            t32 = persist.tile(list(ap.shape), F32, tag=f"dbg{nm}",
                               name=f"dbg{nm}")
            nc.vector.tensor_copy(t32[:], ap)
            nc.sync.dma_start(dbg[nm].ap()[:], t32[:])
        ddump("qTe0", qTe_sb[0][:])
        ddump("qTo0", qTo_sb[0][:])
        ddump("kT0", kT_sb[0][:])
        ddump("v0", v_sb[0][:])
        ddump("v1", v_sb[1][:])
        ddump("sums0", misc_f32[:, 0:8])
        ddump("ot0", ot_sb[0][:])



def _get_nc():
    global _nc_cache
    if _nc_cache is None:
        _nc_cache = _build_kernel()
    return _nc_cache


def _hi_lo(a32):
    hi = a32.astype(E4)
    lo = (a32 - hi.astype(np.float32)).astype(E4)
    return hi, lo


def _pack_w(W):
    # 16x scale keeps the lo residuals out of fp8-subnormal territory
    W32 = np.ascontiguousarray(np.asarray(W, np.float32)) * 16.0
    hi, lo = _hi_lo(W32)
    out = np.empty((P, 2 * NKT * D), E4)
    for part, a in enumerate((hi, lo)):
        for k in range(NKT):
            out[:, (part * NKT + k) * D:(part * NKT + k + 1) * D] = \
                a[k * P:(k + 1) * P, :]
    return out


def _make_masks(chunk0):
    j = np.arange(P)[:, None]
    c = np.arange(P)[None, :]
    curr = (j <= c).astype(ml_dtypes.bfloat16)
    prev = (j > c).astype(ml_dtypes.bfloat16)
    zero = np.zeros_like(curr)
    mask_mid = np.concatenate([prev, prev, curr, curr], axis=1)
    p0 = zero if chunk0 else prev
    mask_q0 = np.concatenate([p0, p0, curr, curr], axis=1)
    ident = np.eye(P, dtype=ml_dtypes.bfloat16)
    ones = np.ones((P, 1), dtype=ml_dtypes.bfloat16)
    sel2 = np.zeros((P, P), dtype=ml_dtypes.bfloat16)
    sel2[0, 0:DH] = 1.0
    sel2[1, DH:P] = 1.0
    return np.concatenate([mask_mid, mask_q0, ident, ones, sel2], axis=1,
                          dtype=ml_dtypes.bfloat16)


def _prep_inputs(x, Wq, Wk, Wv, Wo):
    x = np.asarray(x, np.float32)
    wq8 = _pack_w(Wq)
    wk8 = _pack_w(Wk)
    wv8 = _pack_w(Wv)
    # v carries the 16x Wv scale; fold the inverse into Wo
    wo = (np.asarray(Wo, np.float32) / 16.0).astype(ml_dtypes.bfloat16)
    wo_t = np.empty((P, NKT * D), ml_dtypes.bfloat16)
    for k in range(NKT):
        wo_t[:, k * D:(k + 1) * D] = wo[k * P:(k + 1) * P, :]
    masks_all = _make_masks(False)
    masks_z = _make_masks(True)
    in_maps = []
    for core in range(N_CORES):
        b, chunk = divmod(core, 4)
        c0 = chunk * S
        xh = np.zeros((SH, D), np.float32)
        lo_r = c0 - P
        src_lo = max(0, lo_r)
        xh[src_lo - lo_r:, :] = x[b, src_lo:c0 + S, :]
        xT = np.ascontiguousarray(xh.T)   # [512 dm, 640]
        hi, lo = _hi_lo(xT)
        x8 = np.empty((P, 2 * NKT * SH), E4)
        for part, a in enumerate((hi, lo)):
            for k in range(NKT):
                x8[:, (part * NKT + k) * SH:(part * NKT + k + 1) * SH] = \
                    a[k * P:(k + 1) * P, :]
        in_maps.append({
            "x8": x8, "wq8": wq8, "wk8": wk8, "wv8": wv8, "wo": wo_t,
            "masks": masks_z if chunk == 0 else masks_all,
        })
    return in_maps


def kernel(x, Wq, Wk, Wv, Wo, _profile=None):
    nc = _get_nc()
    in_maps = _prep_inputs(x, Wq, Wk, Wv, Wo)
    res = bass_utils.run_bass_kernel_spmd(nc, in_maps,
                                          core_ids=list(range(N_CORES)))
    x = np.asarray(x)
    B, S_full, _ = x.shape
    out = np.empty((B, S_full, D), np.float32)
    for core in range(N_CORES):
        b, chunk = divmod(core, 4)
        out[b, chunk * S:(chunk + 1) * S, :] = (
            res.results[core]["out"].astype(np.float32))
    if _profile is not None:
        _profile.append(res)
    return out


# revision 4
# speedup vs baseline: 1.0100x; 1.0100x over previous
"""Trainium2 Bass kernel v2: causal sliding-window attention block.

Model (see reference): x:[2,2048,512] -> q/k/v proj (8 heads x 64) ->
causal sliding-window attention (W=128) -> out proj.

Sharding: 8 cores = 2 batches x 4 sequence chunks of 512 rows. Each core
gets a 640-row halo slice of x, all four weight matrices, computes its
512x512 output chunk. No cross-core communication.

v2 redesign vs v1 (driven by the CoreSim-v1 cost model):
  * Projections as fp8e4 DoubleRow matmuls (K=256/instr, 0.5 cyc/row):
    x and W host-pre-split into (hi, lo) fp8 pairs;
    W@x ~= Whi@xhi + Whi@xlo + Wlo@xhi (dropping lo@lo; ~1e-3 rel err,
    better than bf16). 25% fewer PE cycles than bf16.
  * Attention is query-block oriented: per (pair, qblock j) scores tile
    [128w, 512] = [hbj_e | hbj_o | hbj+1_e | hbj+1_o]; one exp, one mask
    multiply, then:
      - softmax denominators via N=1 matmuls (et stationary, ones
        moving) accumulated into a per-pair [128q, 8] PSUM region
        (~free on PE vs 2048 cyc/pair of one-hot matmuls);
      - AV with v stationary / et moving, head halves col-packed into
        one [128, 512] oT bank.
  * Normalize without DRAM bounce: reciprocal of [128, 8] sums (one
    cheap DVE op), PE transposes (identity trick) to [2, 128] rows,
    per-qblock K=2 broadcast matmuls build rbc [128dh, 512q], one DVE
    multiply normalizes the pair.
  * No zero-fill matmuls: PE executes in queue order, so each
    accumulation region's first matmul carries start=True (overwrite).
  * Input DMAs spread across SP/DVE/Pool with host-prepacked
    partition-contiguous layouts; output DMAs spread across engines.
"""

from contextlib import ExitStack

import numpy as np
import ml_dtypes

import concourse.bacc as bacc
import concourse.tile as tile
import concourse.mybir as mybir
from concourse import bass_utils

BF16 = mybir.dt.bfloat16
F32 = mybir.dt.float32
F8 = mybir.dt.float8e4
DRMODE = mybir.MatmulPerfMode.DoubleRow

P = 128          # partitions / block size / window
S = 512          # chunk rows per core
SH = 640         # halo rows per core (128 + 512)
D = 512          # d_model
DH = 64          # head dim
NKT = 4          # d_model tiles of 128
NST = 5          # halo row tiles of 128
NPAIR = 4        # head pairs
N_CORES = 8

E4 = ml_dtypes.float8_e4m3

_nc_cache = None
_DEBUG = False


def _build_kernel():
    nc = bacc.Bacc("TRN2", target_bir_lowering=False, debug=False,
                   enable_asserts=False)

    # x8: [128, 2(hi/lo) x 4(ktile) x 640] fp8; ktile k row p holds
    # x[halo_row, 128k + p].
    x8_d = nc.dram_tensor("x8", [P, 2 * NKT * SH], F8, kind="ExternalInput")
    wq8_d = nc.dram_tensor("wq8", [P, 2 * NKT * D], F8, kind="ExternalInput")
    wk8_d = nc.dram_tensor("wk8", [P, 2 * NKT * D], F8, kind="ExternalInput")
    wv8_d = nc.dram_tensor("wv8", [P, 2 * NKT * D], F8, kind="ExternalInput")
    wo_d = nc.dram_tensor("wo", [P, NKT * D], BF16, kind="ExternalInput")
    # masks blob [128, 1153] bf16:
    #   0:512     mask_mid = [prev|prev|curr|curr]
    #   512:1024  mask_q0  (qblock-0 variant; prev half zeroed on chunk 0)
    #   1024:1152 identity(128)
    #   1152:1153 ones column
    #   1153:1281 sel2 [2, 128] on rows 0:2
    masks_d = nc.dram_tensor("masks", [P, 1281], BF16, kind="ExternalInput")
    out_d = nc.dram_tensor("out", [S, D], BF16, kind="ExternalOutput")
    dbg = {}
    if _DEBUG:
        for nm, shp in [("qTe0", [P, S]), ("qTo0", [P, S]), ("kT0", [P, SH]),
                        ("v0", [P, S]), ("v1", [P, S]), ("et0", [P, S]),
                        ("sums0", [P, 8]), ("ot0", [P, S]), ("rbc0", [P, S]),
                        ("rinv0", [P, 8]), ("trsb0", [2, S]),
                        ("oT0", [P, S])]:
            dbg[nm] = nc.dram_tensor(nm, shp, F32, kind="ExternalOutput")

    with tile.TileContext(nc) as tc, ExitStack() as ctx:
        _kernel_body(ctx, tc, x8_d, wq8_d, wk8_d, wv8_d, wo_d, masks_d, out_d,
                     dbg)
    nc.compile()
    return nc


def _kernel_body(ctx, tc, x8_d, wq8_d, wk8_d, wv8_d, wo_d, masks_d, out_d,
                 dbg={}):
    nc = tc.nc
    Exp = mybir.ActivationFunctionType.Exp
    Copy = mybir.ActivationFunctionType.Copy

    persist = ctx.enter_context(tc.tile_pool(name="persist", bufs=1))
    expp = ctx.enter_context(tc.tile_pool(name="expp", bufs=3))
    outp = ctx.enter_context(tc.tile_pool(name="outp", bufs=4))
    pp = ctx.enter_context(tc.tile_pool(name="pp", bufs=2, space="PSUM"))
    psc = ctx.enter_context(tc.tile_pool(name="psc", bufs=3, space="PSUM"))
    poT = ctx.enter_context(tc.tile_pool(name="poT", bufs=2, space="PSUM"))
    pmisc = ctx.enter_context(tc.tile_pool(name="pmisc", bufs=1, space="PSUM"))

    sel_even = persist.tile([P, 1], F32, tag="sel_even")
    nc.gpsimd.memset(sel_even[:], 1.0)
    nc.gpsimd.memset(sel_even[DH:P, :], 0.0)
    sel_odd = persist.tile([P, 1], F32, tag="sel_odd")
    nc.gpsimd.memset(sel_odd[:], 0.0)
    nc.gpsimd.memset(sel_odd[DH:P, :], 1.0)

    # ---------------- input DMAs (spread across engines) ----------------
    x8 = persist.tile([P, 2 * NKT * SH], F8, tag="x8")
    wq8 = persist.tile([P, 2 * NKT * D], F8, tag="wq8")
    wk8 = persist.tile([P, 2 * NKT * D], F8, tag="wk8")
    wv8 = persist.tile([P, 2 * NKT * D], F8, tag="wv8")
    wo_sb = persist.tile([P, NKT * D], BF16, tag="wo")
    masks = persist.tile([P, 1281], BF16, tag="masks")

    # ACT carries x-hi (its table load is pushed behind this DMA by the
    # dummy activation below); SP: wq-hi then x-lo; Pool: wq-lo, wk, masks.
    nc.scalar.dma_start(x8[:, 0:2 * SH], x8_d.ap()[:, 0:2 * SH])
    nc.sync.dma_start(wq8[:, 0:2 * D], wq8_d.ap()[:, 0:2 * D])
    nc.scalar.dma_start(x8[:, 2 * SH:4 * SH], x8_d.ap()[:, 2 * SH:4 * SH])
    nc.sync.dma_start(wq8[:, 2 * D:NKT * D], wq8_d.ap()[:, 2 * D:NKT * D])
    nc.gpsimd.dma_start(wq8[:, NKT * D:], wq8_d.ap()[:, NKT * D:])
    nc.sync.dma_start(x8[:, 4 * SH:8 * SH], x8_d.ap()[:, 4 * SH:8 * SH])
    nc.gpsimd.dma_start(wk8[:], wk8_d.ap()[:])
    nc.gpsimd.dma_start(masks[:], masks_d.ap()[:])
    nc.sync.dma_start(wv8[:], wv8_d.ap()[:])
    nc.sync.dma_start(wo_sb[:], wo_d.ap()[:])

    # PE warmup: a 1x1 matmul as early as possible latches pe_busy_start
    # so the p-state ramp completes during the input-load stall.
    warm = pmisc.tile([P, 1024], BF16, tag="misc")
    warm_f32 = warm[:].bitcast(F32)
    nc.tensor.matmul(warm_f32[0:1, 40:41], sel_even[0:1, 0:1],
                     sel_even[0:1, 0:1], start=True, stop=True,
                     skip_group_check=True)

    # Dummy activation: the auto-inserted LoadActFuncSet attaches to the
    # first activation and inherits its waits; a no-dep dummy right after
    # the ACT-queue DMA pulls the 1283ns table load off the critical path.
    atl_scratch = persist.tile([P, 1], F32, tag="atl_scratch")
    nc.scalar.activation(atl_scratch[:], sel_even[:], Copy)

    mask_mid = masks[:, 0:512]
    mask_q0 = masks[:, 512:1024]
    ident = masks[:, 1024:1152]
    ones = masks[:, 1152:1153]
    sel2 = masks[0:2, 1153:1281]

    x8v = x8[:].rearrange("p (pt k s) -> p pt k s", pt=2, k=NKT)

    def x_dr(part, kp, c0, c1):
        # [128, 2, c1-c0]: ktile pair (2kp, 2kp+1), halo cols c0:c1
        return x8v[:, part, 2 * kp:2 * kp + 2, c0:c1]

    def w_dr(w8, part, kp, m0, m1):
        wv_ = w8[:].rearrange("p (pt k m) -> p pt k m", pt=2, k=NKT)
        return wv_[:, part, 2 * kp:2 * kp + 2, m0:m1]

    TERMS = [(0, 0), (1, 0), (0, 1)]   # (W part, x part): hi@hi, lo@hi, hi@lo

    def dr_wx(ps_ap, w8, m0, m1, c0, c1):
        """ps = W[:, m0:m1]^T @ xT[:, c0:c1] via 6 DoubleRow matmuls."""
        n = 0
        for wp, xp in TERMS:
            for kp in range(2):
                nc.tensor.matmul(ps_ap, w_dr(w8, wp, kp, m0, m1),
                                 x_dr(xp, kp, c0, c1),
                                 start=(n == 0), stop=(n == 5),
                                 perf_mode=DRMODE)
                n += 1

    def dr_xw(ps_ap, w8, c0, c1):
        """ps = x[c0:c1 rows] @ W via 6 DoubleRow matmuls."""
        n = 0
        for wp, xp in TERMS:
            for kp in range(2):
                nc.tensor.matmul(ps_ap, x_dr(xp, kp, c0, c1),
                                 w_dr(w8, wp, kp, 0, D),
                                 start=(n == 0), stop=(n == 5),
                                 perf_mode=DRMODE)
                n += 1

    # ---------------- projections ----------------
    qTe_sb = [persist.tile([P, S], BF16, tag=f"qTe{t}", name=f"qTe{t}")
              for t in range(NPAIR)]
    qTo_sb = [persist.tile([P, S], BF16, tag=f"qTo{t}", name=f"qTo{t}")
              for t in range(NPAIR)]
    kT_sb = [persist.tile([P, SH], BF16, tag=f"kT{t}", name=f"kT{t}")
             for t in range(NPAIR)]
    v_sb = [persist.tile([P, S], BF16, tag=f"v{st}", name=f"v{st}")
            for st in range(NST)]

    def q_proj(m, pool=None):
        ps = (pool or pp).tile([P, S], F32,
                               tag="pp" if pool is None else "sc",
                               name=f"qps{m}")
        dr_wx(ps[:], wq8, m * P, (m + 1) * P, P, SH)
        nc.scalar.activation(qTe_sb[m][:], ps[:], Copy, scale=sel_even[:])
        nc.vector.tensor_scalar_mul(qTo_sb[m][:], ps[:], sel_odd[:])

    def k_proj(m, pool=None):
        ps = (pool or pp).tile([P, S], F32,
                               tag="pp" if pool is None else "sc",
                               name=f"kps{m}")
        dr_wx(ps[:], wk8, m * P, (m + 1) * P, 0, S)
        nc.scalar.copy(kT_sb[m][:, 0:S], ps[:])
        ps2 = pp.tile([P, S], F32, tag="pp", name=f"kpsh{m}")
        dr_wx(ps2[:, 0:P], wk8, m * P, (m + 1) * P, S, SH)
        nc.vector.tensor_copy(kT_sb[m][:, S:SH], ps2[:, 0:P])

    def v_proj(st):
        ps = pp.tile([P, S], F32, tag="pp", name=f"vps{st}")
        dr_xw(ps[:], wv8, st * P, (st + 1) * P)
        if st % 2 == 0:
            nc.vector.tensor_copy(v_sb[st][:], ps[:])
        else:
            nc.scalar.copy(v_sb[st][:], ps[:])

    # ---------------- attention ----------------
    # misc PSUM bank: bytes 0:128 = sums (4 pairs x [128, 8] f32),
    # bytes 1024:2048 = trT scratch [2, 512] bf16.
    misc = warm
    misc_f32 = warm_f32
    trT = misc[0:2, 512:1024]
    rbcq = misc_f32[:, 128:256]   # [128, 128] f32 slot for pair-3 chainlets

    ot_sb = [persist.tile([P, S], BF16, tag=f"ot{t}", name=f"ot{t}")
             for t in range(NPAIR)]

    def attn_scores(t, j):
        sc = psc.tile([P, S], F32, tag="sc", name=f"sc{t}_{j}")
        for col, (hb, qt) in enumerate(
                [(j, qTe_sb[t]), (j, qTo_sb[t]),
                 (j + 1, qTe_sb[t]), (j + 1, qTo_sb[t])]):
            nc.tensor.matmul(sc[:, col * P:(col + 1) * P],
                             kT_sb[t][:, hb * P:(hb + 1) * P],
                             qt[:, j * P:(j + 1) * P],
                             start=True, stop=True)
        return sc

    def attn_tail(t, j, sc, oT_ps, on_dve):
        et = expp.tile([P, S], BF16, tag="et", name=f"et{t}_{j}")
        # q,k carry a 16x host-side weight scale each: 0.125 / 256
        nc.scalar.activation(et[:], sc[:], Exp, scale=0.125 / 256.0)
        mask = mask_q0 if j == 0 else mask_mid
        nc.gpsimd.tensor_mul(et[:], et[:], mask[:])
        # sums: et slice stationary, ones moving -> [128q, 1] f32
        for h in range(2):
            for bi in range(2):
                c = 8 * t + 2 * j + h
                nc.tensor.matmul(misc_f32[:, c:c + 1],
                                 et[:, (2 * bi + h) * P:(2 * bi + h + 1) * P],
                                 ones[:],
                                 start=(bi == 0), stop=(bi == 1),
                                 skip_group_check=True)
        # AV: v stationary, et moving; head halves col-packed
        for h in range(2):
            for bi in range(2):
                nc.tensor.matmul(
                    oT_ps[h * DH:(h + 1) * DH, j * P:(j + 1) * P],
                    v_sb[j + bi][:, (2 * t + h) * DH:(2 * t + h + 1) * DH],
                    et[:, (2 * bi + h) * P:(2 * bi + h + 1) * P],
                    start=(bi == 0), stop=(bi == 1),
                    tile_position=(0, h * DH), skip_group_check=True)

    def attn_norm(t, oT_ps):
        # oT eviction is independent of the reciprocal chain and frees the
        # poT slot that rbc then reuses; the multiply reads SBUF x PSUM.
        ot_un = persist.tile([P, S], F32, tag="ot_un", name=f"ot_un{t}",
                             bufs=2)
        nc.vector.tensor_copy(ot_un[:], oT_ps[:])
        rinv = persist.tile([P, 8], BF16, tag="rinv", name=f"rinv{t}",
                            bufs=2)
        with nc.allow_low_precision(reason="1/sums in bf16: ~0.4% on the "
                                    "softmax scale, inside error budget"):
            nc.vector.reciprocal(rinv[:], misc_f32[:, 8 * t:8 * t + 8])
        # transpose per qblock: trT[:, j*128:(j+1)*128] = rinv[:, 2j:2j+2]^T
        for j in range(4):
            nc.tensor.transpose(trT[:, j * P:(j + 1) * P],
                                rinv[:, 2 * j:2 * j + 2], ident[:])
        trsb = persist.tile([2, S], BF16, tag="trsb", name=f"trsb{t}",
                            bufs=2)
        nc.vector.tensor_copy(trsb[:], trT[:])
        rbc = sc[t, 3]   # overwrite the pair's last scores tile
        for j in range(4):
            nc.tensor.matmul(rbc[:, j * P:(j + 1) * P],
                             sel2, trsb[:, j * P:(j + 1) * P],
                             start=True, stop=True)
        nc.vector.tensor_mul(ot_sb[t][:], ot_un[:], rbc[:])

    def attn_norm_qb(t, j, ot_un, rbc, rinv, trsb, meng):
        with nc.allow_low_precision(reason="1/sums in bf16, inside budget"):
            nc.vector.reciprocal(rinv[:, 2 * j:2 * j + 2],
                                 misc_f32[:, 8 * t + 2 * j:8 * t + 2 * j + 2])
        nc.tensor.transpose(trT[:, j * P:(j + 1) * P],
                            rinv[:, 2 * j:2 * j + 2], ident[:])
        nc.vector.tensor_copy(trsb[:, j * P:(j + 1) * P],
                              trT[:, j * P:(j + 1) * P])
        nc.tensor.matmul(rbcq, sel2, trsb[:, j * P:(j + 1) * P],
                         start=True, stop=True, skip_group_check=True)
        meng.tensor_mul(ot_sb[t][:, j * P:(j + 1) * P],
                        ot_un[:, j * P:(j + 1) * P],
                        rbcq)

    # ---------------- out projection ----------------
    def fmm(mt, t, f):
        nc.tensor.matmul(f[:], ot_sb[t][:, mt * P:(mt + 1) * P],
                         wo_sb[:, t * D:(t + 1) * D],
                         start=(t == 0), stop=(t == NPAIR - 1),
                         skip_group_check=True)

    def fout(mt, f, ceng, deng):
        osb = outp.tile([P, S], BF16, tag="osb", name=f"osb{mt}")
        if ceng is nc.scalar:
            nc.scalar.copy(osb[:], f[:])
        else:
            ceng.tensor_copy(osb[:], f[:])
        deng.dma_start(out_d.ap()[mt * P:(mt + 1) * P, :], osb[:])

    # ---------------- emission schedule ----------------
    # Flat software pipeline: scores run >=2 qblocks ahead of their
    # sums/AV consumers, with projection work woven into pair 0 and
    # out-projection accumulation woven into pairs 1-3, so the PE queue
    # never parks behind the exp->mask chain.
    q_proj(0)
    q_proj(1)
    k_proj(0, psc)
    q_proj(2, psc)
    q_proj(3, psc)
    v_proj(0)
    v_proj(1)

    fps = {}
    oT = {0: poT.tile([P, S], F32, tag="oT", name="oT0")}
    sc = {(0, 0): attn_scores(0, 0)}
    k_proj(1)
    sc[0, 1] = attn_scores(0, 1)
    attn_tail(0, 0, sc[0, 0], oT[0], False)
    v_proj(2)
    sc[0, 2] = attn_scores(0, 2)
    attn_tail(0, 1, sc[0, 1], oT[0], True)
    k_proj(2)
    v_proj(3)
    sc[0, 3] = attn_scores(0, 3)
    attn_tail(0, 2, sc[0, 2], oT[0], False)
    k_proj(3)
    v_proj(4)
    attn_tail(0, 3, sc[0, 3], oT[0], True)

    for t in (1, 2):
        oT[t] = poT.tile([P, S], F32, tag="oT", name=f"oT{t}")
        sc[t, 0] = attn_scores(t, 0)
        sc[t, 1] = attn_scores(t, 1)
        attn_norm(t - 1, oT[t - 1])   # PE pieces land between score bursts
        attn_tail(t, 0, sc[t, 0], oT[t], False)
        sc[t, 2] = attn_scores(t, 2)
        attn_tail(t, 1, sc[t, 1], oT[t], True)
        if t == 1:
            fps[0] = pp.tile([P, S], F32, tag="pp", name="fps0")
            fps[1] = pp.tile([P, S], F32, tag="pp", name="fps1")
        else:
            for mt in range(2):
                fmm(mt, 0, fps[mt])
        sc[t, 3] = attn_scores(t, 3)
        attn_tail(t, 2, sc[t, 2], oT[t], False)
        attn_tail(t, 3, sc[t, 3], oT[t], True)

    # pair 3: per-qblock normalize chains woven into the attention so the
    # out-projection + output DMA of m-tile j start as soon as qblock j
    # is normalized.
    rinv3 = persist.tile([P, 8], BF16, tag="rinv3", name="rinv3")
    trsb3 = persist.tile([2, S], BF16, tag="trsb3", name="trsb3")
    ot_un3 = persist.tile([P, S], F32, tag="ot_un", name="ot_un3", bufs=2)
    oT[3] = poT.tile([P, S], F32, tag="oT", name="oT3")
    sc[3, 0] = attn_scores(3, 0)
    sc[3, 1] = attn_scores(3, 1)
    attn_norm(2, oT[2])
    attn_tail(3, 0, sc[3, 0], oT[3], False)
    sc[3, 2] = attn_scores(3, 2)
    attn_tail(3, 1, sc[3, 1], oT[3], True)
    for mt in range(2):
        fmm(mt, 1, fps[mt])
    fps[2] = psc.tile([P, S], F32, tag="sc", name="fps2")
    for tt in range(3):
        fmm(2, tt, fps[2])
    nc.vector.tensor_copy(ot_un3[:, 0:P], oT[3][:, 0:P])
    attn_norm_qb(3, 0, ot_un3, None, rinv3, trsb3, nc.vector)
    sc[3, 3] = attn_scores(3, 3)
    attn_tail(3, 2, sc[3, 2], oT[3], False)
    nc.scalar.copy(ot_un3[:, P:2 * P], oT[3][:, P:2 * P])
    fps[3] = psc.tile([P, S], F32, tag="sc", name="fps3")
    for tt in range(3):
        fmm(3, tt, fps[3])
    for mt in range(2):
        fmm(mt, 2, fps[mt])
    attn_norm_qb(3, 1, ot_un3, None, rinv3, trsb3, nc.vector)
    fmm(0, 3, fps[0])
    fout(0, fps[0], nc.vector, nc.sync)
    attn_tail(3, 3, sc[3, 3], oT[3], True)
    nc.vector.tensor_copy(ot_un3[:, 2 * P:3 * P], oT[3][:, 2 * P:3 * P])
    nc.scalar.copy(ot_un3[:, 3 * P:4 * P], oT[3][:, 3 * P:4 * P])
    fmm(1, 3, fps[1])
    fout(1, fps[1], nc.scalar, nc.scalar)
    attn_norm_qb(3, 2, ot_un3, None, rinv3, trsb3, nc.vector)
    fmm(2, 3, fps[2])
    fout(2, fps[2], nc.scalar, nc.gpsimd)
    attn_norm_qb(3, 3, ot_un3, None, rinv3, trsb3, nc.vector)
    fmm(3, 3, fps[3])
    fout(3, fps[3], nc.vector, nc.sync)

    if dbg:
        def ddump(nm, ap):
            t32 = persist.tile(list(ap.shape), F32, tag=f"dbg{nm}",
                               name=f"dbg{nm}")
            nc.vector.tensor_copy(t32[:], ap)
            nc.sync.dma_start(dbg[nm].ap()[:], t32[:])
        ddump("qTe0", qTe_sb[0][:])
        ddump("qTo0", qTo_sb[0][:])
        ddump("kT0", kT_sb[0][:])
        ddump("v0", v_sb[0][:])
        ddump("v1", v_sb[1][:])
        ddump("sums0", misc_f32[:, 0:8])
        ddump("ot0", ot_sb[0][:])



def _get_nc():
    global _nc_cache
    if _nc_cache is None:
        _nc_cache = _build_kernel()
    return _nc_cache


def _hi_lo(a32):
    hi = a32.astype(E4)
    lo = (a32 - hi.astype(np.float32)).astype(E4)
    return hi, lo


def _pack_w(W):
    # 16x scale keeps the lo residuals out of fp8-subnormal territory
    W32 = np.ascontiguousarray(np.asarray(W, np.float32)) * 16.0
    hi, lo = _hi_lo(W32)
    out = np.empty((P, 2 * NKT * D), E4)
    for part, a in enumerate((hi, lo)):
        for k in range(NKT):
            out[:, (part * NKT + k) * D:(part * NKT + k + 1) * D] = \
                a[k * P:(k + 1) * P, :]
    return out


def _make_masks(chunk0):
    j = np.arange(P)[:, None]
    c = np.arange(P)[None, :]
    curr = (j <= c).astype(ml_dtypes.bfloat16)
    prev = (j > c).astype(ml_dtypes.bfloat16)
    zero = np.zeros_like(curr)
    mask_mid = np.concatenate([prev, prev, curr, curr], axis=1)
    p0 = zero if chunk0 else prev
    mask_q0 = np.concatenate([p0, p0, curr, curr], axis=1)
    ident = np.eye(P, dtype=ml_dtypes.bfloat16)
    ones = np.ones((P, 1), dtype=ml_dtypes.bfloat16)
    sel2 = np.zeros((P, P), dtype=ml_dtypes.bfloat16)
    sel2[0, 0:DH] = 1.0
    sel2[1, DH:P] = 1.0
    return np.concatenate([mask_mid, mask_q0, ident, ones, sel2], axis=1,
                          dtype=ml_dtypes.bfloat16)


def _prep_inputs(x, Wq, Wk, Wv, Wo):
    x = np.asarray(x, np.float32)
    wq8 = _pack_w(Wq)
    wk8 = _pack_w(Wk)
    wv8 = _pack_w(Wv)
    # v carries the 16x Wv scale; fold the inverse into Wo
    wo = (np.asarray(Wo, np.float32) / 16.0).astype(ml_dtypes.bfloat16)
    wo_t = np.empty((P, NKT * D), ml_dtypes.bfloat16)
    for k in range(NKT):
        wo_t[:, k * D:(k + 1) * D] = wo[k * P:(k + 1) * P, :]
    masks_all = _make_masks(False)
    masks_z = _make_masks(True)
    in_maps = []
    for core in range(N_CORES):
        b, chunk = divmod(core, 4)
        c0 = chunk * S
        xh = np.zeros((SH, D), np.float32)
        lo_r = c0 - P
        src_lo = max(0, lo_r)
        xh[src_lo - lo_r:, :] = x[b, src_lo:c0 + S, :]
        xT = np.ascontiguousarray(xh.T)   # [512 dm, 640]
        hi, lo = _hi_lo(xT)
        x8 = np.empty((P, 2 * NKT * SH), E4)
        for part, a in enumerate((hi, lo)):
            for k in range(NKT):
                x8[:, (part * NKT + k) * SH:(part * NKT + k + 1) * SH] = \
                    a[k * P:(k + 1) * P, :]
        in_maps.append({
            "x8": x8, "wq8": wq8, "wk8": wk8, "wv8": wv8, "wo": wo_t,
            "masks": masks_z if chunk == 0 else masks_all,
        })
    return in_maps


def kernel(x, Wq, Wk, Wv, Wo, _profile=None):
    nc = _get_nc()
    in_maps = _prep_inputs(x, Wq, Wk, Wv, Wo)
    res = bass_utils.run_bass_kernel_spmd(nc, in_maps,
                                          core_ids=list(range(N_CORES)))
    x = np.asarray(x)
    B, S_full, _ = x.shape
    out = np.empty((B, S_full, D), np.float32)
    for core in range(N_CORES):
        b, chunk = divmod(core, 4)
        out[b, chunk * S:(chunk + 1) * S, :] = (
            res.results[core]["out"].astype(np.float32))
    if _profile is not None:
        _profile.append(res)
    return out
